# revision 32
# baseline (speedup 1.0000x reference)
"""Trainium2 Bass kernel for the spiking autoencoder (nn_AE_spikes).

Data-parallel across 8 NeuronCores: each core gets 1024 samples.

Algorithm (validated host-side against the reference):
  inp   = floor(16*features)/16                       (layer-0 input, constant over T)
  c0    = inp @ W0.T                                  (hoisted out of the time loop)
  v_l   = b_l  (pre-reset membrane total);  sls3 = 0
  for t in 16:
     v0 = reset(v0) + c0          f0 = (v0>=1)        # reset(v) = v if v<1 else 0
     v1 = reset(v1) + W1@f0       f1 = (v1>=1)
     v2 = reset(v2) + W2@f1       f2 = (v2>=1)
     v3 = reset(v3) + W3@f2       sls3 += (v3>=1)
  out = sls3 / 16
(The reference's `out[tot==bias]=0` quirk requires an exact f32 equality that is
measure-zero on this data; it is dropped — validated to produce 0 mismatches.)

Implementation notes:
 - Neurons on partitions, batch on free dim; 784 padded to 896 = 7*128.
 - The LIF reset is one fused DVE scalar_tensor_tensor: v = (v<1)*v.
 - Spike masks are produced on ScalarE (exact Sign + affine remap to 0/1),
   keeping VectorE (the bottleneck) to ~2 passes per element per step.
 - Layer-3 spike counting accumulates Sign values in bf16 at DVE 2x rate;
   the epilogue maps sls = (sum_sign + 16)/2 and scales by 1/16 in one op.
 - Matmuls run in bf16 with a two-term weight split (W ~ hi + lo); spike
   inputs are exactly representable in bf16 so the only error is the split
   residual (~1e-6 relative), far below the spike-threshold margins.
"""

import numpy as np

HIDDEN = 128
IN = 784
PAD = 896          # 7 * 128
NG = 7             # partition groups for the 784/896-sized dims
T = 16
BATCH = 8192
NCORES = 8
BLOC = BATCH // NCORES       # 1024 samples per core
CH = 256                     # batch chunk (matmul free dim)
NCH = BLOC // CH             # 4 chunks per core, two resident in PSUM at a time

_CACHE = {}


def _patch_tile_drain():
    """This walrus build accepts at most ONE semaphore wait per instruction.
    Tile's tail drain stuffs every outstanding proc's wait onto a single Drain;
    split them across no-fuse NOPs instead (one wait each)."""
    import concourse.tile as tile_mod
    import concourse.mybir as mybir
    from concourse.vector_clock import ScopedClock

    if getattr(tile_mod.TileContext, "_drain_split_patched", False):
        return

    def _drain_and_barrier_split(self, tick_clock, wait_clock):
        probe = self.nc.sync.nop(nofuse=True, hint="drain_wait_probe")
        wait_clock.add_sem_waits(probe.ins, ScopedClock({None: tick_clock.global_clock}))
        si = probe.ins.sync_info
        waits = list(si.on_wait) if si is not None else []
        if si is not None:
            si.on_wait = waits[:1]
        for w in waits[1:]:
            nop = self.nc.sync.nop(nofuse=True, hint="drain_wait_extra")
            nop.ins.sync_info = mybir.SyncInfo(on_update=[], on_wait=[w])
        self.nc.sync.drain()
        self.nc.all_engine_barrier()
        assert self.sems is not None
        popped = self.nc._tile_sem_poison_stack.pop()
        assert popped is self._sem_poison
        self.nc.clear_and_free_semaphores(list(self.sems.allocated().values()))
        self.nc.all_engine_barrier()

    tile_mod.TileContext._drain_and_barrier = _drain_and_barrier_split
    tile_mod.TileContext._drain_split_patched = True


def _legalize_waits(nc):
    """This walrus build accepts at most one semaphore wait per instruction
    (two on EventSemaphore).  Hoist excess waits onto injected EventSemaphore
    carrier instructions placed immediately before, on the same engine."""
    import concourse.mybir as mybir

    n_carrier = 0
    for f in nc.m.functions:
        for bb in f.blocks:
            insts = bb.instructions
            new = []
            changed = False
            for inst in insts:
                si = inst.sync_info
                waits = list(si.on_wait) if si is not None and si.on_wait else []
                cap = 2 if isinstance(inst, mybir.InstEventSemaphore) else 1
                if len(waits) > cap:
                    changed = True
                    keep = waits[-cap:]
                    extra = waits[:-cap]
                    while extra:
                        pair, extra = extra[:2], extra[2:]
                        carrier = mybir.InstEventSemaphore(
                            name=f"wsplit_{n_carrier}", ins=[], outs=[])
                        n_carrier += 1
                        carrier.engine = inst.engine
                        carrier.sync_info = mybir.SyncInfo(on_update=[], on_wait=pair)
                        new.append(carrier)
                    si.on_wait = keep
                new.append(inst)
            if changed:
                bb.instructions[:] = new
    return n_carrier


def _build_bass():
    import concourse.bass as bass
    import concourse.mybir as mybir
    from concourse.tile import TileContext
    from contextlib import ExitStack

    _patch_tile_drain()

    dt = mybir.dt
    Alu = mybir.AluOpType
    BF = dt.bfloat16

    nc = bass.Bass()

    # ---- DRAM I/O (per-core shard views) ----
    xt = nc.dram_tensor("xt", [PAD, BLOC], dt.float32, kind="ExternalInput")
    w0h = nc.dram_tensor("w0h", [PAD, HIDDEN], BF, kind="ExternalInput")
    w0l = nc.dram_tensor("w0l", [PAD, HIDDEN], BF, kind="ExternalInput")
    w1h = nc.dram_tensor("w1h", [HIDDEN, HIDDEN], BF, kind="ExternalInput")
    w1l = nc.dram_tensor("w1l", [HIDDEN, HIDDEN], BF, kind="ExternalInput")
    w2h = nc.dram_tensor("w2h", [HIDDEN, HIDDEN], BF, kind="ExternalInput")
    w2l = nc.dram_tensor("w2l", [HIDDEN, HIDDEN], BF, kind="ExternalInput")
    w3h = nc.dram_tensor("w3h", [HIDDEN, PAD], BF, kind="ExternalInput")
    w3l = nc.dram_tensor("w3l", [HIDDEN, PAD], BF, kind="ExternalInput")
    b0 = nc.dram_tensor("b0", [HIDDEN, 1], dt.float32, kind="ExternalInput")
    b1 = nc.dram_tensor("b1", [HIDDEN, 1], dt.float32, kind="ExternalInput")
    b2 = nc.dram_tensor("b2", [HIDDEN, 1], dt.float32, kind="ExternalInput")
    b3r = nc.dram_tensor("b3r", [1, PAD], dt.float32, kind="ExternalInput")
    out_d = nc.dram_tensor("out", [PAD, BLOC], dt.float32, kind="ExternalOutput")

    es = ExitStack()
    with es:
        tc = es.enter_context(TileContext(nc))

        wpool = es.enter_context(tc.tile_pool(name="wpool", bufs=1))
        cpool = es.enter_context(tc.tile_pool(name="cpool", bufs=1))
        spool = es.enter_context(tc.tile_pool(name="spool", bufs=1))
        psum3p = es.enter_context(tc.tile_pool(name="psum3p", bufs=1, space="PSUM"))

        # ---- weights / consts to SBUF ----
        w0_sb = []  # [term][group] -> [128,128] bf16
        for nm, dram in (("w0h", w0h), ("w0l", w0l)):
            tile = wpool.tile([HIDDEN, NG * HIDDEN], BF, name=f"{nm}_sb")
            for c in range(NG):
                nc.scalar.dma_start(out=tile[:, c * HIDDEN:(c + 1) * HIDDEN],
                                    in_=dram[c * HIDDEN:(c + 1) * HIDDEN, :])
            w0_sb.append(tile)
        w12_sb = []  # [(w1h,w1l),(w2h,w2l)]
        for nm, dram in (("w1h", w1h), ("w1l", w1l), ("w2h", w2h), ("w2l", w2l)):
            tile = wpool.tile([HIDDEN, HIDDEN], BF, name=f"{nm}_sb")
            nc.sync.dma_start(out=tile[:], in_=dram[:])
            w12_sb.append(tile)
        w3_sb = []
        for nm, dram in (("w3h", w3h), ("w3l", w3l)):
            tile = wpool.tile([HIDDEN, NG * HIDDEN], BF, name=f"{nm}_sb")
            nc.sync.dma_start(out=tile[:], in_=dram[:])
            w3_sb.append(tile)

        b_sb = []
        for nm, dram in (("b0", b0), ("b1", b1), ("b2", b2)):
            tile = cpool.tile([HIDDEN, 1], dt.float32, name=f"{nm}_sb")
            nc.sync.dma_start(out=tile[:], in_=dram[:])
            b_sb.append(tile)
        b3row = cpool.tile([1, PAD], dt.float32)
        nc.sync.dma_start(out=b3row[:], in_=b3r[:])
        ones_row = cpool.tile([1, CH], dt.float32)
        nc.vector.memset(ones_row[:], 1.0)

        zeros = cpool.tile([HIDDEN, CH], dt.float32)
        nc.vector.memset(zeros[:], 0.0)

        # ---- load + quantize input: xq <- floor(16 * x) as bf16 ----
        # HW f32->int32 cast rounds to nearest; rint(16x - (0.5 - 2^-20)) ==
        # floor(16x) exactly on the 2^-19 input grid.  Per k-group on ScalarE
        # so the casts pipeline with the input DMAs and stay off VectorE.
        xq_sb = spool.tile([HIDDEN, NG * BLOC], BF)
        with tc.tile_pool(name="qpool", bufs=2) as qpool:
            for c in range(NG):
                x_sb = qpool.tile([HIDDEN, BLOC], dt.float32, name="x_sb", tag="xg")
                dma_eng = nc.sync if c % 2 == 0 else nc.scalar
                dma_eng.dma_start(out=x_sb[:], in_=xt[c * HIDDEN:(c + 1) * HIDDEN, :])
                xi_sb = qpool.tile([HIDDEN, BLOC], dt.int32, name="xi_sb", tag="xi")
                nc.vector.tensor_scalar(xi_sb[:], x_sb[:], 16.0, -0.4999990463256836,
                                        Alu.mult, Alu.add)
                nc.scalar.activation(xq_sb[:, c * BLOC:(c + 1) * BLOC], xi_sb[:],
                                     mybir.ActivationFunctionType.Copy,
                                     bias=0.0, scale=1.0)


        # ---- state ----
        # Four 256-sample chunks; two resident at a time in ONE [128,4096] PSUM
        # mega-tile (8 banks): chunk A's v3 = cols 0:1792, chunk B's v3 =
        # cols 1792:3584, and cols 3584:3840 (bank 7) are the shared
        # c0/mm1/mm2 scratch.  W3 matmuls accumulate the layer-3 integrate in
        # place.  start=True clears has_written BANK-WIDE, so only the first
        # matmul ever touching each of banks 0-6 uses start=True (the t=0 b3
        # seeds, emitted in bank order; the mega-tile makes Tile's bank
        # tracker keep same-bank program order), and the scratch (whose bank
        # holds nothing else) is cleared by each use's leading start=True.
        L3W = NG * CH                               # 1792
        v012 = spool.tile([HIDDEN, NCH * 3 * CH], dt.float32)
        def vl(l, ch):
            off = ch * 3 * CH + l * CH
            return v012[:, off:off + CH]
        ssum3 = [spool.tile([HIDDEN, L3W], BF, name=f"ssum3_{ch}") for ch in range(NCH)]
        c0 = spool.tile([HIDDEN, BLOC], dt.float32)
        f0 = [spool.tile([HIDDEN, CH], BF, name=f"f0_{ch}") for ch in range(NCH)]
        f1 = [spool.tile([HIDDEN, CH], BF, name=f"f1_{ch}") for ch in range(NCH)]
        f2 = [spool.tile([HIDDEN, CH], BF, name=f"f2_{ch}") for ch in range(NCH)]
        sg0 = [spool.tile([HIDDEN, CH], BF, name=f"sg0_{ch}") for ch in range(NCH)]
        sg1 = [spool.tile([HIDDEN, CH], BF, name=f"sg1_{ch}") for ch in range(NCH)]
        sg2 = [spool.tile([HIDDEN, CH], BF, name=f"sg2_{ch}") for ch in range(NCH)]
        sg3 = [[spool.tile([HIDDEN, L3W], BF, name=f"sg3_{ch}_{p}") for p in range(2)]
               for ch in range(NCH)]
        inv3 = [spool.tile([HIDDEN, L3W], BF, name=f"inv3_{ch}") for ch in range(NCH)]
        outb = [spool.tile([HIDDEN, L3W], dt.float32, name=f"outb_{ch}") for ch in range(NCH)]
        mone = cpool.tile([HIDDEN, 1], dt.float32)
        nc.vector.memset(mone[:], -1.0)

        Sign = mybir.ActivationFunctionType.Sign
        Copy = mybir.ActivationFunctionType.Copy

        mega = psum3p.tile([HIDDEN, 4096], dt.float32, name="mega", tag="mega")
        scr = mega[:, 3584:3584 + CH]
        def v3base(ch):
            return 0 if ch % 2 == 0 else L3W
        def v3ap(ch, lo=0, hi=L3W):
            return mega[:, v3base(ch) + lo:v3base(ch) + hi]

        # c0 for all four chunks up front (through the scratch bank), so a
        # pair boundary only waits on the b3 seeds
        for ch in range(NCH):
            n = 0
            for c in range(NG):
                for term in range(2):
                    nc.tensor.matmul(scr,
                                     w0_sb[term][:, c * HIDDEN:(c + 1) * HIDDEN],
                                     xq_sb[:, c * BLOC + ch * CH: c * BLOC + ch * CH + CH],
                                     start=(n == 0), stop=(n == 2 * NG - 1))
                    n += 1
            nc.scalar.copy(c0[:, ch * CH:(ch + 1) * CH], scr)

        for pair in range(NCH // 2):
            chunks = (2 * pair, 2 * pair + 1)
            vbase = {chunks[0]: 0, chunks[1]: L3W}
            for ch in chunks:
                for l in range(3):
                    nc.vector.tensor_scalar(vl(l, ch), zeros[:, 0:CH],
                                            b_sb[l][:, 0:1], None, Alu.add)
                nc.vector.memset(ssum3[ch][:], 0.0)

            for t in range(T):
                for ch in chunks:
                    vch = v012[:, ch * 3 * CH:(ch + 1) * 3 * CH]
                    nc.vector.scalar_tensor_tensor(vch, vch, 1.0, vch, Alu.is_lt, Alu.mult)
                for ch in chunks:
                    nc.vector.tensor_tensor(vl(0, ch), vl(0, ch),
                                            c0[:, ch * CH:(ch + 1) * CH], Alu.add)
                    nc.scalar.activation(sg0[ch][:], vl(0, ch), Sign, bias=mone[:, 0:1], scale=1.0)
                    nc.scalar.activation(f0[ch][:], sg0[ch][:], Copy, bias=0.5, scale=0.5)
                for ch in chunks:
                    nc.tensor.matmul(scr, w12_sb[0][:], f0[ch][:], start=True, stop=False)
                    nc.tensor.matmul(scr, w12_sb[1][:], f0[ch][:], start=False, stop=True)
                    nc.vector.tensor_tensor(vl(1, ch), vl(1, ch), scr, Alu.add)
                    nc.scalar.activation(sg1[ch][:], vl(1, ch), Sign, bias=mone[:, 0:1], scale=1.0)
                    nc.scalar.activation(f1[ch][:], sg1[ch][:], Copy, bias=0.5, scale=0.5)
                for ch in chunks:
                    nc.tensor.matmul(scr, w12_sb[2][:], f1[ch][:], start=True, stop=False)
                    nc.tensor.matmul(scr, w12_sb[3][:], f1[ch][:], start=False, stop=True)
                    nc.vector.tensor_tensor(vl(2, ch), vl(2, ch), scr, Alu.add)
                    nc.scalar.activation(sg2[ch][:], vl(2, ch), Sign, bias=mone[:, 0:1], scale=1.0)
                    nc.vector.tensor_scalar(f2[ch][:], sg2[ch][:], 0.5, 0.5, Alu.mult, Alu.add)
                for ch in chunks:
                    for c in range(NG):
                        sl = v3ap(ch, c * CH, (c + 1) * CH)
                        if t == 0:
                            # b3 seed; start=True only on the first region of
                            # each bank (cols % 512 == 0 within the mega-tile)
                            bank_first = ((vbase[ch] + c * CH) % 512 == 0)
                            nc.tensor.matmul(sl, b3row[0:1, c * HIDDEN:(c + 1) * HIDDEN],
                                             ones_row[0:1, :], start=bank_first, stop=False)
                        nc.tensor.matmul(sl, w3_sb[0][:, c * HIDDEN:(c + 1) * HIDDEN],
                                         f2[ch][:], start=False, stop=False)
                        nc.tensor.matmul(sl, w3_sb[1][:, c * HIDDEN:(c + 1) * HIDDEN],
                                         f2[ch][:], start=False, stop=(t == T - 1))
                    sg3t = sg3[ch][t % 2]
                    nc.scalar.activation(sg3t[:], v3ap(ch), Sign,
                                         bias=mone[:, 0:1], scale=1.0)
                    nc.vector.tensor_tensor(ssum3[ch][:], ssum3[ch][:], sg3t[:], Alu.add)
                    if t < T - 1:
                        nc.vector.tensor_scalar(inv3[ch][:], sg3t[:], -0.5, 0.5,
                                                Alu.mult, Alu.add)
                        nc.vector.tensor_tensor(v3ap(ch), v3ap(ch), inv3[ch][:], Alu.mult)

            for ch in chunks:
                nc.vector.tensor_scalar(outb[ch][:], ssum3[ch][:], 16.0, 1.0 / 32.0,
                                        Alu.add, Alu.mult)
                for c in range(NG):
                    dma_eng = nc.sync if c % 2 == 0 else nc.scalar
                    dma_eng.dma_start(out=out_d[c * HIDDEN:(c + 1) * HIDDEN,
                                                ch * CH:(ch + 1) * CH],
                                      in_=outb[ch][:, c * CH:(c + 1) * CH])

    _legalize_waits(nc)
    return nc


def _bf16_pair(wT):
    """Return (hi, lo) bf16 arrays with hi + lo ~= wT (f32)."""
    import ml_dtypes
    bf = ml_dtypes.bfloat16
    hi = wT.astype(bf)
    lo = (wT - hi.astype(np.float32)).astype(bf)
    return hi, lo


def _prep_shards(features, W0, b0, W1, b1, W2, b2, W3, b3):
    """Host-side layout prep: shard batch, transpose to [neuron, batch], pad to 896."""
    f32 = np.float32
    w0t = np.zeros((PAD, HIDDEN), f32)
    w0t[:IN, :] = (np.asarray(W0, f32) / 16.0).T
    w1t = np.ascontiguousarray(np.asarray(W1, f32).T)
    w2t = np.ascontiguousarray(np.asarray(W2, f32).T)
    w3t = np.zeros((HIDDEN, PAD), f32)
    w3t[:, :IN] = np.asarray(W3, f32).T
    b3p = np.zeros((PAD,), f32)
    b3p[:IN] = np.asarray(b3, f32)
    b3r = b3p.reshape(1, PAD)
    w0hi, w0lo = _bf16_pair(w0t)
    w1hi, w1lo = _bf16_pair(w1t)
    w2hi, w2lo = _bf16_pair(w2t)
    w3hi, w3lo = _bf16_pair(w3t)
    common = {
        "w0h": w0hi, "w0l": w0lo, "w1h": w1hi, "w1l": w1lo,
        "w2h": w2hi, "w2l": w2lo, "w3h": w3hi, "w3l": w3lo,
        "b0": np.asarray(b0, f32).reshape(HIDDEN, 1),
        "b1": np.asarray(b1, f32).reshape(HIDDEN, 1),
        "b2": np.asarray(b2, f32).reshape(HIDDEN, 1),
        "b3r": b3r,
    }
    in_maps = []
    feats = np.asarray(features, f32)
    for i in range(NCORES):
        shard = feats[i * BLOC:(i + 1) * BLOC]          # [1024, 784]
        xt = np.zeros((PAD, BLOC), f32)
        xt[:IN, :] = shard.T
        in_maps.append({"xt": np.ascontiguousarray(xt), **common})
    return in_maps


def _run(in_maps, **kw):
    from concourse.bass_utils import run_bass_kernel_spmd

    if "nc" not in _CACHE:
        _CACHE["nc"] = _build_bass()
    return run_bass_kernel_spmd(_CACHE["nc"], in_maps, core_ids=list(range(NCORES)), **kw)


def kernel(features, W0, b0, W1, b1, W2, b2, W3, b3):
    in_maps = _prep_shards(features, W0, b0, W1, b1, W2, b2, W3, b3)
    res = _run(in_maps)
    out = np.empty((BATCH, IN), np.float32)
    for i in range(NCORES):
        out[i * BLOC:(i + 1) * BLOC, :] = res.results[i]["out"][:IN, :].T
    return out


def bench(inputs, trace=True, tmpdir=None):
    """Run with NTFF profiling; returns BassKernelResults (exec_time_ns etc)."""
    in_maps = _prep_shards(**inputs)
    return _run(in_maps, trace=trace, tmpdir=tmpdir)


# revision 33
# speedup vs baseline: 1.1362x; 1.1362x over previous
"""Trainium2 Bass kernel for the spiking autoencoder (nn_AE_spikes).

Data-parallel across 8 NeuronCores: each core gets 1024 samples.

Algorithm (validated host-side against the reference):
  inp   = floor(16*features)/16                       (layer-0 input, constant over T)
  c0    = inp @ W0.T                                  (hoisted out of the time loop)
  v_l   = b_l  (pre-reset membrane total);  sls3 = 0
  for t in 16:
     v0 = reset(v0) + c0          f0 = (v0>=1)        # reset(v) = v if v<1 else 0
     v1 = reset(v1) + W1@f0       f1 = (v1>=1)
     v2 = reset(v2) + W2@f1       f2 = (v2>=1)
     v3 = reset(v3) + W3@f2       sls3 += (v3>=1)
  out = sls3 / 16
(The reference's `out[tot==bias]=0` quirk requires an exact f32 equality that is
measure-zero on this data; it is dropped — validated to produce 0 mismatches.)

Implementation notes:
 - Neurons on partitions, batch on free dim; 784 padded to 896 = 7*128.
 - The LIF reset is one fused DVE scalar_tensor_tensor: v = (v<1)*v.
 - Spike masks are produced on ScalarE (exact Sign + affine remap to 0/1),
   keeping VectorE (the bottleneck) to ~2 passes per element per step.
 - Layer-3 spike counting accumulates Sign values in bf16 at DVE 2x rate;
   the epilogue maps sls = (sum_sign + 16)/2 and scales by 1/16 in one op.
 - Matmuls run in bf16 with a two-term weight split (W ~ hi + lo); spike
   inputs are exactly representable in bf16 so the only error is the split
   residual (~1e-6 relative), far below the spike-threshold margins.
"""

import numpy as np

HIDDEN = 128
IN = 784
PAD = 896          # 7 * 128
NG = 7             # partition groups for the 784/896-sized dims
T = 16
BATCH = 8192
NCORES = 8
BLOC = BATCH // NCORES       # 1024 samples per core
CH = 256                     # batch chunk (matmul free dim)
NCH = BLOC // CH             # 4 chunks per core, two resident in PSUM at a time

_CACHE = {}


def _patch_tile_drain():
    """This walrus build accepts at most ONE semaphore wait per instruction.
    Tile's tail drain stuffs every outstanding proc's wait onto a single Drain;
    split them across no-fuse NOPs instead (one wait each)."""
    import concourse.tile as tile_mod
    import concourse.mybir as mybir
    from concourse.vector_clock import ScopedClock

    if getattr(tile_mod.TileContext, "_drain_split_patched", False):
        return

    def _drain_and_barrier_split(self, tick_clock, wait_clock):
        probe = self.nc.sync.nop(nofuse=True, hint="drain_wait_probe")
        wait_clock.add_sem_waits(probe.ins, ScopedClock({None: tick_clock.global_clock}))
        si = probe.ins.sync_info
        waits = list(si.on_wait) if si is not None else []
        if si is not None:
            si.on_wait = waits[:1]
        for w in waits[1:]:
            nop = self.nc.sync.nop(nofuse=True, hint="drain_wait_extra")
            nop.ins.sync_info = mybir.SyncInfo(on_update=[], on_wait=[w])
        self.nc.sync.drain()
        self.nc.all_engine_barrier()
        assert self.sems is not None
        popped = self.nc._tile_sem_poison_stack.pop()
        assert popped is self._sem_poison
        self.nc.clear_and_free_semaphores(list(self.sems.allocated().values()))
        self.nc.all_engine_barrier()

    tile_mod.TileContext._drain_and_barrier = _drain_and_barrier_split
    tile_mod.TileContext._drain_split_patched = True


def _legalize_waits(nc):
    """This walrus build accepts at most one semaphore wait per instruction
    (two on EventSemaphore).  Hoist excess waits onto injected EventSemaphore
    carrier instructions placed immediately before, on the same engine."""
    import concourse.mybir as mybir

    n_carrier = 0
    for f in nc.m.functions:
        for bb in f.blocks:
            insts = bb.instructions
            new = []
            changed = False
            for inst in insts:
                si = inst.sync_info
                waits = list(si.on_wait) if si is not None and si.on_wait else []
                cap = 2 if isinstance(inst, mybir.InstEventSemaphore) else 1
                if len(waits) > cap:
                    changed = True
                    keep = waits[-cap:]
                    extra = waits[:-cap]
                    while extra:
                        pair, extra = extra[:2], extra[2:]
                        carrier = mybir.InstEventSemaphore(
                            name=f"wsplit_{n_carrier}", ins=[], outs=[])
                        n_carrier += 1
                        carrier.engine = inst.engine
                        carrier.sync_info = mybir.SyncInfo(on_update=[], on_wait=pair)
                        new.append(carrier)
                    si.on_wait = keep
                new.append(inst)
            if changed:
                bb.instructions[:] = new
    return n_carrier


def _build_bass():
    import concourse.bass as bass
    import concourse.mybir as mybir
    from concourse.tile import TileContext
    from contextlib import ExitStack

    _patch_tile_drain()

    dt = mybir.dt
    Alu = mybir.AluOpType
    BF = dt.bfloat16

    nc = bass.Bass()

    # ---- DRAM I/O (per-core shard views) ----
    xt = nc.dram_tensor("xt", [PAD, BLOC], dt.float32, kind="ExternalInput")
    w0h = nc.dram_tensor("w0h", [PAD, HIDDEN], BF, kind="ExternalInput")
    w0l = nc.dram_tensor("w0l", [PAD, HIDDEN], BF, kind="ExternalInput")
    w1h = nc.dram_tensor("w1h", [HIDDEN, HIDDEN], BF, kind="ExternalInput")
    w1l = nc.dram_tensor("w1l", [HIDDEN, HIDDEN], BF, kind="ExternalInput")
    w2h = nc.dram_tensor("w2h", [HIDDEN, HIDDEN], BF, kind="ExternalInput")
    w2l = nc.dram_tensor("w2l", [HIDDEN, HIDDEN], BF, kind="ExternalInput")
    w3h = nc.dram_tensor("w3h", [HIDDEN, PAD], BF, kind="ExternalInput")
    w3l = nc.dram_tensor("w3l", [HIDDEN, PAD], BF, kind="ExternalInput")
    b0 = nc.dram_tensor("b0", [HIDDEN, 1], dt.float32, kind="ExternalInput")
    b1 = nc.dram_tensor("b1", [HIDDEN, 1], dt.float32, kind="ExternalInput")
    b2 = nc.dram_tensor("b2", [HIDDEN, 1], dt.float32, kind="ExternalInput")
    b3r = nc.dram_tensor("b3r", [1, PAD], dt.float32, kind="ExternalInput")
    out_d = nc.dram_tensor("out", [PAD, BLOC], dt.float32, kind="ExternalOutput")

    es = ExitStack()
    with es:
        tc = es.enter_context(TileContext(nc))

        wpool = es.enter_context(tc.tile_pool(name="wpool", bufs=1))
        cpool = es.enter_context(tc.tile_pool(name="cpool", bufs=1))
        spool = es.enter_context(tc.tile_pool(name="spool", bufs=1))
        psum3p = es.enter_context(tc.tile_pool(name="psum3p", bufs=1, space="PSUM"))

        # ---- weights / consts to SBUF ----
        w0_sb = []  # [term][group] -> [128,128] bf16
        for nm, dram in (("w0h", w0h), ("w0l", w0l)):
            tile = wpool.tile([HIDDEN, NG * HIDDEN], BF, name=f"{nm}_sb")
            for c in range(NG):
                nc.scalar.dma_start(out=tile[:, c * HIDDEN:(c + 1) * HIDDEN],
                                    in_=dram[c * HIDDEN:(c + 1) * HIDDEN, :])
            w0_sb.append(tile)
        w12_sb = []  # [(w1h,w1l),(w2h,w2l)]
        for nm, dram in (("w1h", w1h), ("w1l", w1l), ("w2h", w2h), ("w2l", w2l)):
            tile = wpool.tile([HIDDEN, HIDDEN], BF, name=f"{nm}_sb")
            nc.sync.dma_start(out=tile[:], in_=dram[:])
            w12_sb.append(tile)
        w3_sb = []
        for nm, dram in (("w3h", w3h), ("w3l", w3l)):
            tile = wpool.tile([HIDDEN, NG * HIDDEN], BF, name=f"{nm}_sb")
            nc.sync.dma_start(out=tile[:], in_=dram[:])
            w3_sb.append(tile)

        b_sb = []
        for nm, dram in (("b0", b0), ("b1", b1), ("b2", b2)):
            tile = cpool.tile([HIDDEN, 1], dt.float32, name=f"{nm}_sb")
            nc.sync.dma_start(out=tile[:], in_=dram[:])
            b_sb.append(tile)
        b3row = cpool.tile([1, PAD], dt.float32)
        nc.sync.dma_start(out=b3row[:], in_=b3r[:])
        ones_row = cpool.tile([1, CH], dt.float32)
        nc.vector.memset(ones_row[:], 1.0)

        zeros = cpool.tile([HIDDEN, CH], dt.float32)
        nc.vector.memset(zeros[:], 0.0)

        # ---- load + quantize input: xq <- floor(16 * x) as bf16 ----
        # HW f32->int32 cast rounds to nearest; rint(16x - (0.5 - 2^-20)) ==
        # floor(16x) exactly on the 2^-19 input grid.  Per k-group on ScalarE
        # so the casts pipeline with the input DMAs and stay off VectorE.
        xq_sb = spool.tile([HIDDEN, NG * BLOC], BF)
        with tc.tile_pool(name="qpool", bufs=2) as qpool:
            for c in range(NG):
                x_sb = qpool.tile([HIDDEN, BLOC], dt.float32, name="x_sb", tag="xg")
                dma_eng = nc.sync if c % 2 == 0 else nc.scalar
                dma_eng.dma_start(out=x_sb[:], in_=xt[c * HIDDEN:(c + 1) * HIDDEN, :])
                xi_sb = qpool.tile([HIDDEN, BLOC], dt.int32, name="xi_sb", tag="xi")
                nc.vector.tensor_scalar(xi_sb[:], x_sb[:], 16.0, -0.4999990463256836,
                                        Alu.mult, Alu.add)
                nc.scalar.activation(xq_sb[:, c * BLOC:(c + 1) * BLOC], xi_sb[:],
                                     mybir.ActivationFunctionType.Copy,
                                     bias=0.0, scale=1.0)


        # ---- state ----
        # Four 256-sample chunks; two resident at a time in ONE [128,4096] PSUM
        # mega-tile (8 banks): chunk A's v3 = cols 0:1792, chunk B's v3 =
        # cols 1792:3584, and cols 3584:3840 (bank 7) are the shared
        # c0/mm1/mm2 scratch.  W3 matmuls accumulate the layer-3 integrate in
        # place.  start=True clears has_written BANK-WIDE, so only the first
        # matmul ever touching each of banks 0-6 uses start=True (the t=0 b3
        # seeds, emitted in bank order; the mega-tile makes Tile's bank
        # tracker keep same-bank program order), and the scratch (whose bank
        # holds nothing else) is cleared by each use's leading start=True.
        L3W = NG * CH                               # 1792
        v012 = spool.tile([HIDDEN, NCH * 3 * CH], dt.float32)
        def vl(l, ch):
            off = ch * 3 * CH + l * CH
            return v012[:, off:off + CH]
        ssum3 = [spool.tile([HIDDEN, L3W], BF, name=f"ssum3_{ch}") for ch in range(NCH)]
        c0 = spool.tile([HIDDEN, BLOC], dt.float32)
        f0 = [spool.tile([HIDDEN, CH], BF, name=f"f0_{ch}") for ch in range(NCH)]
        f1 = [spool.tile([HIDDEN, CH], BF, name=f"f1_{ch}") for ch in range(NCH)]
        f2 = [spool.tile([HIDDEN, CH], BF, name=f"f2_{ch}") for ch in range(NCH)]
        sg0 = [spool.tile([HIDDEN, CH], BF, name=f"sg0_{ch}") for ch in range(NCH)]
        sg1 = [spool.tile([HIDDEN, CH], BF, name=f"sg1_{ch}") for ch in range(NCH)]
        sg2 = [spool.tile([HIDDEN, CH], BF, name=f"sg2_{ch}") for ch in range(NCH)]
        sg3 = [[spool.tile([HIDDEN, L3W], BF, name=f"sg3_{ch}_{p}") for p in range(2)]
               for ch in range(NCH)]
        inv3 = [spool.tile([HIDDEN, L3W], BF, name=f"inv3_{ch}") for ch in range(NCH)]
        outb = [spool.tile([HIDDEN, L3W], dt.float32, name=f"outb_{ch}") for ch in range(NCH)]
        mone = cpool.tile([HIDDEN, 1], dt.float32)
        nc.vector.memset(mone[:], -1.0)

        Sign = mybir.ActivationFunctionType.Sign
        Copy = mybir.ActivationFunctionType.Copy

        mega = psum3p.tile([HIDDEN, 4096], dt.float32, name="mega", tag="mega")
        scr = mega[:, 3584:3584 + CH]
        def v3base(ch):
            return 0 if ch % 2 == 0 else L3W
        def v3ap(ch, lo=0, hi=L3W):
            return mega[:, v3base(ch) + lo:v3base(ch) + hi]

        # c0 for all four chunks up front (through the scratch bank), so a
        # pair boundary only waits on the b3 seeds
        for ch in range(NCH):
            n = 0
            for c in range(NG):
                for term in range(2):
                    nc.tensor.matmul(scr,
                                     w0_sb[term][:, c * HIDDEN:(c + 1) * HIDDEN],
                                     xq_sb[:, c * BLOC + ch * CH: c * BLOC + ch * CH + CH],
                                     start=(n == 0), stop=(n == 2 * NG - 1))
                    n += 1
            nc.scalar.copy(c0[:, ch * CH:(ch + 1) * CH], scr)

        for pair in range(NCH // 2):
            chunks = (2 * pair, 2 * pair + 1)
            vbase = {chunks[0]: 0, chunks[1]: L3W}
            for ch in chunks:
                for l in range(3):
                    nc.vector.tensor_scalar(vl(l, ch), zeros[:, 0:CH],
                                            b_sb[l][:, 0:1], None, Alu.add)
                nc.vector.memset(ssum3[ch][:], 0.0)

            for t in range(T):
                for ch in chunks:
                    vch = v012[:, ch * 3 * CH:(ch + 1) * 3 * CH]
                    nc.vector.scalar_tensor_tensor(vch, vch, 1.0, vch, Alu.is_lt, Alu.mult)
                for ch in chunks:
                    nc.vector.tensor_tensor(vl(0, ch), vl(0, ch),
                                            c0[:, ch * CH:(ch + 1) * CH], Alu.add)
                    nc.scalar.activation(sg0[ch][:], vl(0, ch), Sign, bias=mone[:, 0:1], scale=1.0)
                    nc.scalar.activation(f0[ch][:], sg0[ch][:], Copy, bias=0.5, scale=0.5)
                for ch in chunks:
                    nc.tensor.matmul(scr, w12_sb[0][:], f0[ch][:], start=True, stop=False)
                    nc.tensor.matmul(scr, w12_sb[1][:], f0[ch][:], start=False, stop=True)
                    nc.vector.tensor_tensor(vl(1, ch), vl(1, ch), scr, Alu.add)
                    nc.scalar.activation(sg1[ch][:], vl(1, ch), Sign, bias=mone[:, 0:1], scale=1.0)
                    nc.scalar.activation(f1[ch][:], sg1[ch][:], Copy, bias=0.5, scale=0.5)
                for ch in chunks:
                    nc.tensor.matmul(scr, w12_sb[2][:], f1[ch][:], start=True, stop=False)
                    nc.tensor.matmul(scr, w12_sb[3][:], f1[ch][:], start=False, stop=True)
                    nc.vector.tensor_tensor(vl(2, ch), vl(2, ch), scr, Alu.add)
                    nc.scalar.activation(sg2[ch][:], vl(2, ch), Sign, bias=mone[:, 0:1], scale=1.0)
                    nc.scalar.activation(f2[ch][:], sg2[ch][:], Copy, bias=0.5, scale=0.5)
                for ch in chunks:
                    for c in range(NG):
                        sl = v3ap(ch, c * CH, (c + 1) * CH)
                        if t == 0:
                            # b3 seed; start=True only on the first region of
                            # each bank (cols % 512 == 0 within the mega-tile)
                            bank_first = ((vbase[ch] + c * CH) % 512 == 0)
                            nc.tensor.matmul(sl, b3row[0:1, c * HIDDEN:(c + 1) * HIDDEN],
                                             ones_row[0:1, :], start=bank_first, stop=False)
                        nc.tensor.matmul(sl, w3_sb[0][:, c * HIDDEN:(c + 1) * HIDDEN],
                                         f2[ch][:], start=False, stop=False)
                        nc.tensor.matmul(sl, w3_sb[1][:, c * HIDDEN:(c + 1) * HIDDEN],
                                         f2[ch][:], start=False, stop=(t == T - 1))
                    sg3t = sg3[ch][t % 2]
                    nc.scalar.activation(sg3t[:], v3ap(ch), Sign,
                                         bias=mone[:, 0:1], scale=1.0)
                    nc.vector.tensor_tensor(ssum3[ch][:], ssum3[ch][:], sg3t[:], Alu.add)
                    if t < T - 1:
                        nc.vector.tensor_scalar(inv3[ch][:], sg3t[:], -0.5, 0.5,
                                                Alu.mult, Alu.add)
                        nc.vector.tensor_tensor(v3ap(ch), v3ap(ch), inv3[ch][:], Alu.mult)

            for ch in chunks:
                nc.vector.tensor_scalar(outb[ch][:], ssum3[ch][:], 16.0, 1.0 / 32.0,
                                        Alu.add, Alu.mult)
                for c in range(NG):
                    dma_eng = nc.sync if c % 2 == 0 else nc.scalar
                    dma_eng.dma_start(out=out_d[c * HIDDEN:(c + 1) * HIDDEN,
                                                ch * CH:(ch + 1) * CH],
                                      in_=outb[ch][:, c * CH:(c + 1) * CH])

    _legalize_waits(nc)
    return nc


def _bf16_pair(wT):
    """Return (hi, lo) bf16 arrays with hi + lo ~= wT (f32)."""
    import ml_dtypes
    bf = ml_dtypes.bfloat16
    hi = wT.astype(bf)
    lo = (wT - hi.astype(np.float32)).astype(bf)
    return hi, lo


def _prep_shards(features, W0, b0, W1, b1, W2, b2, W3, b3):
    """Host-side layout prep: shard batch, transpose to [neuron, batch], pad to 896."""
    f32 = np.float32
    w0t = np.zeros((PAD, HIDDEN), f32)
    w0t[:IN, :] = (np.asarray(W0, f32) / 16.0).T
    w1t = np.ascontiguousarray(np.asarray(W1, f32).T)
    w2t = np.ascontiguousarray(np.asarray(W2, f32).T)
    w3t = np.zeros((HIDDEN, PAD), f32)
    w3t[:, :IN] = np.asarray(W3, f32).T
    b3p = np.zeros((PAD,), f32)
    b3p[:IN] = np.asarray(b3, f32)
    b3r = b3p.reshape(1, PAD)
    w0hi, w0lo = _bf16_pair(w0t)
    w1hi, w1lo = _bf16_pair(w1t)
    w2hi, w2lo = _bf16_pair(w2t)
    w3hi, w3lo = _bf16_pair(w3t)
    common = {
        "w0h": w0hi, "w0l": w0lo, "w1h": w1hi, "w1l": w1lo,
        "w2h": w2hi, "w2l": w2lo, "w3h": w3hi, "w3l": w3lo,
        "b0": np.asarray(b0, f32).reshape(HIDDEN, 1),
        "b1": np.asarray(b1, f32).reshape(HIDDEN, 1),
        "b2": np.asarray(b2, f32).reshape(HIDDEN, 1),
        "b3r": b3r,
    }
    in_maps = []
    feats = np.asarray(features, f32)
    for i in range(NCORES):
        shard = feats[i * BLOC:(i + 1) * BLOC]          # [1024, 784]
        xt = np.zeros((PAD, BLOC), f32)
        xt[:IN, :] = shard.T
        in_maps.append({"xt": np.ascontiguousarray(xt), **common})
    return in_maps


def _run(in_maps, **kw):
    from concourse.bass_utils import run_bass_kernel_spmd

    if "nc" not in _CACHE:
        _CACHE["nc"] = _build_bass()
    return run_bass_kernel_spmd(_CACHE["nc"], in_maps, core_ids=list(range(NCORES)), **kw)


def kernel(features, W0, b0, W1, b1, W2, b2, W3, b3):
    in_maps = _prep_shards(features, W0, b0, W1, b1, W2, b2, W3, b3)
    res = _run(in_maps)
    out = np.empty((BATCH, IN), np.float32)
    for i in range(NCORES):
        out[i * BLOC:(i + 1) * BLOC, :] = res.results[i]["out"][:IN, :].T
    return out


def bench(inputs, trace=True, tmpdir=None):
    """Run with NTFF profiling; returns BassKernelResults (exec_time_ns etc)."""
    in_maps = _prep_shards(**inputs)
    return _run(in_maps, trace=trace, tmpdir=tmpdir)


# revision 34
# speedup vs baseline: 1.1602x; 1.0211x over previous
"""Trainium2 Bass kernel for the spiking autoencoder (nn_AE_spikes).

Data-parallel across 8 NeuronCores: each core gets 1024 samples.

Algorithm (validated host-side against the reference):
  inp   = floor(16*features)/16                       (layer-0 input, constant over T)
  c0    = inp @ W0.T                                  (hoisted out of the time loop)
  v_l   = b_l  (pre-reset membrane total);  sls3 = 0
  for t in 16:
     v0 = reset(v0) + c0          f0 = (v0>=1)        # reset(v) = v if v<1 else 0
     v1 = reset(v1) + W1@f0       f1 = (v1>=1)
     v2 = reset(v2) + W2@f1       f2 = (v2>=1)
     v3 = reset(v3) + W3@f2       sls3 += (v3>=1)
  out = sls3 / 16
(The reference's `out[tot==bias]=0` quirk requires an exact f32 equality that is
measure-zero on this data; it is dropped — validated to produce 0 mismatches.)

Implementation notes:
 - Neurons on partitions, batch on free dim; 784 padded to 896 = 7*128.
 - The LIF reset is one fused DVE scalar_tensor_tensor: v = (v<1)*v.
 - Spike masks are produced on ScalarE (exact Sign + affine remap to 0/1),
   keeping VectorE (the bottleneck) to ~2 passes per element per step.
 - Layer-3 spike counting accumulates Sign values in bf16 at DVE 2x rate;
   the epilogue maps sls = (sum_sign + 16)/2 and scales by 1/16 in one op.
 - Matmuls run in bf16 with a two-term weight split (W ~ hi + lo); spike
   inputs are exactly representable in bf16 so the only error is the split
   residual (~1e-6 relative), far below the spike-threshold margins.
"""

import numpy as np

HIDDEN = 128
IN = 784
PAD = 896          # 7 * 128
NG = 7             # partition groups for the 784/896-sized dims
T = 16
BATCH = 8192
NCORES = 8
BLOC = BATCH // NCORES       # 1024 samples per core
CH = 256                     # batch chunk (matmul free dim)
NCH = BLOC // CH             # 4 chunks per core, two resident in PSUM at a time

_CACHE = {}


def _patch_tile_drain():
    """This walrus build accepts at most ONE semaphore wait per instruction.
    Tile's tail drain stuffs every outstanding proc's wait onto a single Drain;
    split them across no-fuse NOPs instead (one wait each)."""
    import concourse.tile as tile_mod
    import concourse.mybir as mybir
    from concourse.vector_clock import ScopedClock

    if getattr(tile_mod.TileContext, "_drain_split_patched", False):
        return

    def _drain_and_barrier_split(self, tick_clock, wait_clock):
        probe = self.nc.sync.nop(nofuse=True, hint="drain_wait_probe")
        wait_clock.add_sem_waits(probe.ins, ScopedClock({None: tick_clock.global_clock}))
        si = probe.ins.sync_info
        waits = list(si.on_wait) if si is not None else []
        if si is not None:
            si.on_wait = waits[:1]
        for w in waits[1:]:
            nop = self.nc.sync.nop(nofuse=True, hint="drain_wait_extra")
            nop.ins.sync_info = mybir.SyncInfo(on_update=[], on_wait=[w])
        self.nc.sync.drain()
        self.nc.all_engine_barrier()
        assert self.sems is not None
        popped = self.nc._tile_sem_poison_stack.pop()
        assert popped is self._sem_poison
        self.nc.clear_and_free_semaphores(list(self.sems.allocated().values()))
        self.nc.all_engine_barrier()

    tile_mod.TileContext._drain_and_barrier = _drain_and_barrier_split
    tile_mod.TileContext._drain_split_patched = True


def _legalize_waits(nc):
    """This walrus build accepts at most one semaphore wait per instruction
    (two on EventSemaphore).  Hoist excess waits onto injected EventSemaphore
    carrier instructions placed immediately before, on the same engine."""
    import concourse.mybir as mybir

    n_carrier = 0
    for f in nc.m.functions:
        for bb in f.blocks:
            insts = bb.instructions
            new = []
            changed = False
            for inst in insts:
                si = inst.sync_info
                waits = list(si.on_wait) if si is not None and si.on_wait else []
                cap = 2 if isinstance(inst, mybir.InstEventSemaphore) else 1
                if len(waits) > cap:
                    changed = True
                    keep = waits[-cap:]
                    extra = waits[:-cap]
                    while extra:
                        pair, extra = extra[:2], extra[2:]
                        carrier = mybir.InstEventSemaphore(
                            name=f"wsplit_{n_carrier}", ins=[], outs=[])
                        n_carrier += 1
                        carrier.engine = inst.engine
                        carrier.sync_info = mybir.SyncInfo(on_update=[], on_wait=pair)
                        new.append(carrier)
                    si.on_wait = keep
                new.append(inst)
            if changed:
                bb.instructions[:] = new
    return n_carrier


def _build_bass():
    import concourse.bass as bass
    import concourse.mybir as mybir
    from concourse.tile import TileContext
    from contextlib import ExitStack

    _patch_tile_drain()

    dt = mybir.dt
    Alu = mybir.AluOpType
    BF = dt.bfloat16

    nc = bass.Bass()

    # ---- DRAM I/O (per-core shard views) ----
    xt = nc.dram_tensor("xt", [PAD, BLOC], dt.float32, kind="ExternalInput")
    w0h = nc.dram_tensor("w0h", [PAD, HIDDEN], BF, kind="ExternalInput")
    w0l = nc.dram_tensor("w0l", [PAD, HIDDEN], BF, kind="ExternalInput")
    w1h = nc.dram_tensor("w1h", [HIDDEN, HIDDEN], BF, kind="ExternalInput")
    w1l = nc.dram_tensor("w1l", [HIDDEN, HIDDEN], BF, kind="ExternalInput")
    w2h = nc.dram_tensor("w2h", [HIDDEN, HIDDEN], BF, kind="ExternalInput")
    w2l = nc.dram_tensor("w2l", [HIDDEN, HIDDEN], BF, kind="ExternalInput")
    w3h = nc.dram_tensor("w3h", [HIDDEN, PAD], BF, kind="ExternalInput")
    w3l = nc.dram_tensor("w3l", [HIDDEN, PAD], BF, kind="ExternalInput")
    b0 = nc.dram_tensor("b0", [HIDDEN, 1], dt.float32, kind="ExternalInput")
    b1 = nc.dram_tensor("b1", [HIDDEN, 1], dt.float32, kind="ExternalInput")
    b2 = nc.dram_tensor("b2", [HIDDEN, 1], dt.float32, kind="ExternalInput")
    b3r = nc.dram_tensor("b3r", [1, PAD], dt.float32, kind="ExternalInput")
    out_d = nc.dram_tensor("out", [PAD, BLOC], dt.float32, kind="ExternalOutput")

    es = ExitStack()
    with es:
        tc = es.enter_context(TileContext(nc))

        wpool = es.enter_context(tc.tile_pool(name="wpool", bufs=1))
        cpool = es.enter_context(tc.tile_pool(name="cpool", bufs=1))
        spool = es.enter_context(tc.tile_pool(name="spool", bufs=1))
        psum3p = es.enter_context(tc.tile_pool(name="psum3p", bufs=1, space="PSUM"))

        # ---- weights / consts to SBUF ----
        w0_sb = []  # [term][group] -> [128,128] bf16
        for nm, dram in (("w0h", w0h), ("w0l", w0l)):
            tile = wpool.tile([HIDDEN, NG * HIDDEN], BF, name=f"{nm}_sb")
            for c in range(NG):
                nc.scalar.dma_start(out=tile[:, c * HIDDEN:(c + 1) * HIDDEN],
                                    in_=dram[c * HIDDEN:(c + 1) * HIDDEN, :])
            w0_sb.append(tile)
        w12_sb = []  # [(w1h,w1l),(w2h,w2l)]
        for nm, dram in (("w1h", w1h), ("w1l", w1l), ("w2h", w2h), ("w2l", w2l)):
            tile = wpool.tile([HIDDEN, HIDDEN], BF, name=f"{nm}_sb")
            nc.sync.dma_start(out=tile[:], in_=dram[:])
            w12_sb.append(tile)
        w3_sb = []
        for nm, dram in (("w3h", w3h), ("w3l", w3l)):
            tile = wpool.tile([HIDDEN, NG * HIDDEN], BF, name=f"{nm}_sb")
            nc.sync.dma_start(out=tile[:], in_=dram[:])
            w3_sb.append(tile)

        b_sb = []
        for nm, dram in (("b0", b0), ("b1", b1), ("b2", b2)):
            tile = cpool.tile([HIDDEN, 1], dt.float32, name=f"{nm}_sb")
            nc.sync.dma_start(out=tile[:], in_=dram[:])
            b_sb.append(tile)
        b3row = cpool.tile([1, PAD], dt.float32)
        nc.sync.dma_start(out=b3row[:], in_=b3r[:])
        ones_row = cpool.tile([1, CH], dt.float32)
        nc.vector.memset(ones_row[:], 1.0)

        zeros = cpool.tile([HIDDEN, CH], dt.float32)
        nc.vector.memset(zeros[:], 0.0)

        # ---- load + quantize input: xq <- floor(16 * x) as bf16 ----
        # HW f32->int32 cast rounds to nearest; rint(16x - (0.5 - 2^-20)) ==
        # floor(16x) exactly on the 2^-19 input grid.  Per k-group on ScalarE
        # so the casts pipeline with the input DMAs and stay off VectorE.
        xq_sb = spool.tile([HIDDEN, NG * BLOC], BF)
        with tc.tile_pool(name="qpool", bufs=2) as qpool:
            for c in range(NG):
                x_sb = qpool.tile([HIDDEN, BLOC], dt.float32, name="x_sb", tag="xg")
                dma_eng = nc.sync if c % 2 == 0 else nc.scalar
                dma_eng.dma_start(out=x_sb[:], in_=xt[c * HIDDEN:(c + 1) * HIDDEN, :])
                xi_sb = qpool.tile([HIDDEN, BLOC], dt.int32, name="xi_sb", tag="xi")
                nc.vector.tensor_scalar(xi_sb[:], x_sb[:], 16.0, -0.4999990463256836,
                                        Alu.mult, Alu.add)
                nc.scalar.activation(xq_sb[:, c * BLOC:(c + 1) * BLOC], xi_sb[:],
                                     mybir.ActivationFunctionType.Copy,
                                     bias=0.0, scale=1.0)


        # ---- state ----
        # Four 256-sample chunks; two resident at a time in ONE [128,4096] PSUM
        # mega-tile (8 banks): chunk A's v3 = cols 0:1792, chunk B's v3 =
        # cols 1792:3584, and cols 3584:3840 (bank 7) are the shared
        # c0/mm1/mm2 scratch.  W3 matmuls accumulate the layer-3 integrate in
        # place.  start=True clears has_written BANK-WIDE, so only the first
        # matmul ever touching each of banks 0-6 uses start=True (the t=0 b3
        # seeds, emitted in bank order; the mega-tile makes Tile's bank
        # tracker keep same-bank program order), and the scratch (whose bank
        # holds nothing else) is cleared by each use's leading start=True.
        L3W = NG * CH                               # 1792
        v012 = spool.tile([HIDDEN, NCH * 3 * CH], dt.float32)
        def vl(l, ch):
            off = ch * 3 * CH + l * CH
            return v012[:, off:off + CH]
        ssum3 = [spool.tile([HIDDEN, L3W], BF, name=f"ssum3_{ch}") for ch in range(NCH)]
        c0 = spool.tile([HIDDEN, BLOC], dt.float32)
        f0 = [spool.tile([HIDDEN, CH], BF, name=f"f0_{ch}") for ch in range(NCH)]
        f1 = [spool.tile([HIDDEN, CH], BF, name=f"f1_{ch}") for ch in range(NCH)]
        f2 = [spool.tile([HIDDEN, CH], BF, name=f"f2_{ch}") for ch in range(NCH)]
        sg3 = [[spool.tile([HIDDEN, L3W], BF, name=f"sg3_{ch}_{p}") for p in range(2)]
               for ch in range(NCH)]
        inv3 = [spool.tile([HIDDEN, L3W], BF, name=f"inv3_{ch}") for ch in range(NCH)]
        outb = [spool.tile([HIDDEN, L3W], dt.float32, name=f"outb_{ch}") for ch in range(NCH)]
        mone = cpool.tile([HIDDEN, 1], dt.float32)
        nc.vector.memset(mone[:], -1.0)

        Sign = mybir.ActivationFunctionType.Sign
        Copy = mybir.ActivationFunctionType.Copy

        mega = psum3p.tile([HIDDEN, 4096], dt.float32, name="mega", tag="mega")
        scr = mega[:, 3584:3584 + CH]
        def v3base(ch):
            return 0 if ch % 2 == 0 else L3W
        def v3ap(ch, lo=0, hi=L3W):
            return mega[:, v3base(ch) + lo:v3base(ch) + hi]

        # c0 for all four chunks up front (through the scratch bank), so a
        # pair boundary only waits on the b3 seeds
        for ch in range(NCH):
            n = 0
            for c in range(NG):
                for term in range(2):
                    nc.tensor.matmul(scr,
                                     w0_sb[term][:, c * HIDDEN:(c + 1) * HIDDEN],
                                     xq_sb[:, c * BLOC + ch * CH: c * BLOC + ch * CH + CH],
                                     start=(n == 0), stop=(n == 2 * NG - 1))
                    n += 1
            nc.scalar.copy(c0[:, ch * CH:(ch + 1) * CH], scr)

        for pair in range(NCH // 2):
            chunks = (2 * pair, 2 * pair + 1)
            vbase = {chunks[0]: 0, chunks[1]: L3W}
            for ch in chunks:
                for l in range(3):
                    nc.vector.tensor_scalar(vl(l, ch), zeros[:, 0:CH],
                                            b_sb[l][:, 0:1], None, Alu.add)
                nc.vector.memset(ssum3[ch][:], 0.0)

            for t in range(T):
                for ch in chunks:
                    vch = v012[:, ch * 3 * CH:(ch + 1) * 3 * CH]
                    nc.vector.scalar_tensor_tensor(vch, vch, 1.0, vch, Alu.is_lt, Alu.mult)
                for ch in chunks:
                    nc.vector.tensor_tensor(vl(0, ch), vl(0, ch),
                                            c0[:, ch * CH:(ch + 1) * CH], Alu.add)
                    nc.vector.tensor_scalar(f0[ch][:], vl(0, ch), 1.0, None, Alu.is_ge)
                for ch in chunks:
                    nc.tensor.matmul(scr, w12_sb[0][:], f0[ch][:], start=True, stop=False)
                    nc.tensor.matmul(scr, w12_sb[1][:], f0[ch][:], start=False, stop=True)
                    nc.vector.tensor_tensor(vl(1, ch), vl(1, ch), scr, Alu.add)
                    nc.vector.tensor_scalar(f1[ch][:], vl(1, ch), 1.0, None, Alu.is_ge)
                for ch in chunks:
                    nc.tensor.matmul(scr, w12_sb[2][:], f1[ch][:], start=True, stop=False)
                    nc.tensor.matmul(scr, w12_sb[3][:], f1[ch][:], start=False, stop=True)
                    nc.vector.tensor_tensor(vl(2, ch), vl(2, ch), scr, Alu.add)
                    nc.vector.tensor_scalar(f2[ch][:], vl(2, ch), 1.0, None, Alu.is_ge)
                for ch in chunks:
                    for c in range(NG):
                        sl = v3ap(ch, c * CH, (c + 1) * CH)
                        if t == 0:
                            # b3 seed; start=True only on the first region of
                            # each bank (cols % 512 == 0 within the mega-tile)
                            bank_first = ((vbase[ch] + c * CH) % 512 == 0)
                            nc.tensor.matmul(sl, b3row[0:1, c * HIDDEN:(c + 1) * HIDDEN],
                                             ones_row[0:1, :], start=bank_first, stop=False)
                        nc.tensor.matmul(sl, w3_sb[0][:, c * HIDDEN:(c + 1) * HIDDEN],
                                         f2[ch][:], start=False, stop=False)
                        nc.tensor.matmul(sl, w3_sb[1][:, c * HIDDEN:(c + 1) * HIDDEN],
                                         f2[ch][:], start=False, stop=(t == T - 1))
                    sg3t = sg3[ch][t % 2]
                    nc.scalar.activation(sg3t[:], v3ap(ch), Sign,
                                         bias=mone[:, 0:1], scale=1.0)
                    nc.vector.tensor_tensor(ssum3[ch][:], ssum3[ch][:], sg3t[:], Alu.add)
                    if t < T - 1:
                        nc.vector.tensor_scalar(inv3[ch][:], sg3t[:], -0.5, 0.5,
                                                Alu.mult, Alu.add)
                        nc.vector.tensor_tensor(v3ap(ch), v3ap(ch), inv3[ch][:], Alu.mult)

            for ch in chunks:
                nc.vector.tensor_scalar(outb[ch][:], ssum3[ch][:], 16.0, 1.0 / 32.0,
                                        Alu.add, Alu.mult)
                for c in range(NG):
                    dma_eng = nc.sync if c % 2 == 0 else nc.scalar
                    dma_eng.dma_start(out=out_d[c * HIDDEN:(c + 1) * HIDDEN,
                                                ch * CH:(ch + 1) * CH],
                                      in_=outb[ch][:, c * CH:(c + 1) * CH])

    _legalize_waits(nc)
    return nc


def _bf16_pair(wT):
    """Return (hi, lo) bf16 arrays with hi + lo ~= wT (f32)."""
    import ml_dtypes
    bf = ml_dtypes.bfloat16
    hi = wT.astype(bf)
    lo = (wT - hi.astype(np.float32)).astype(bf)
    return hi, lo


def _prep_shards(features, W0, b0, W1, b1, W2, b2, W3, b3):
    """Host-side layout prep: shard batch, transpose to [neuron, batch], pad to 896."""
    f32 = np.float32
    w0t = np.zeros((PAD, HIDDEN), f32)
    w0t[:IN, :] = (np.asarray(W0, f32) / 16.0).T
    w1t = np.ascontiguousarray(np.asarray(W1, f32).T)
    w2t = np.ascontiguousarray(np.asarray(W2, f32).T)
    w3t = np.zeros((HIDDEN, PAD), f32)
    w3t[:, :IN] = np.asarray(W3, f32).T
    b3p = np.zeros((PAD,), f32)
    b3p[:IN] = np.asarray(b3, f32)
    b3r = b3p.reshape(1, PAD)
    w0hi, w0lo = _bf16_pair(w0t)
    w1hi, w1lo = _bf16_pair(w1t)
    w2hi, w2lo = _bf16_pair(w2t)
    w3hi, w3lo = _bf16_pair(w3t)
    common = {
        "w0h": w0hi, "w0l": w0lo, "w1h": w1hi, "w1l": w1lo,
        "w2h": w2hi, "w2l": w2lo, "w3h": w3hi, "w3l": w3lo,
        "b0": np.asarray(b0, f32).reshape(HIDDEN, 1),
        "b1": np.asarray(b1, f32).reshape(HIDDEN, 1),
        "b2": np.asarray(b2, f32).reshape(HIDDEN, 1),
        "b3r": b3r,
    }
    in_maps = []
    feats = np.asarray(features, f32)
    for i in range(NCORES):
        shard = feats[i * BLOC:(i + 1) * BLOC]          # [1024, 784]
        xt = np.zeros((PAD, BLOC), f32)
        xt[:IN, :] = shard.T
        in_maps.append({"xt": np.ascontiguousarray(xt), **common})
    return in_maps


def _run(in_maps, **kw):
    from concourse.bass_utils import run_bass_kernel_spmd

    if "nc" not in _CACHE:
        _CACHE["nc"] = _build_bass()
    return run_bass_kernel_spmd(_CACHE["nc"], in_maps, core_ids=list(range(NCORES)), **kw)


def kernel(features, W0, b0, W1, b1, W2, b2, W3, b3):
    in_maps = _prep_shards(features, W0, b0, W1, b1, W2, b2, W3, b3)
    res = _run(in_maps)
    out = np.empty((BATCH, IN), np.float32)
    for i in range(NCORES):
        out[i * BLOC:(i + 1) * BLOC, :] = res.results[i]["out"][:IN, :].T
    return out


def bench(inputs, trace=True, tmpdir=None):
    """Run with NTFF profiling; returns BassKernelResults (exec_time_ns etc)."""
    in_maps = _prep_shards(**inputs)
    return _run(in_maps, trace=trace, tmpdir=tmpdir)
